# revision 23
# baseline (speedup 1.0000x reference)
"""Trainium2 Bass kernel for CombinedLoss (focal + dice + boundary-weighted BCE).

Contract: kernel(inputs, targets) takes FULL (64,1,512,512) fp32 arrays and
returns the full scalar loss (fp32). Data-parallel over batch: 8 images per
NeuronCore on 8 cores; host combines per-core partial sums in float64.

Design (engine-balanced, transposed layout [128 p = row-in-band,
32 slots = img*4+band, 512 cols], bf16/fp8 inputs):
  Host sends x (bf16), t2 = 32*(t-0.5) (bf16, exact), and m (t zero-padded
  2 cols each side, fp8e4).
  zh = t2*x (DVE TT, 2x bf16 mode); q = exp(-zh/16); bce = ln(1+q) +acc;
  pt = exp(-bce) +acc  (ScalarE, single pinned exp/ln table set).
  FOCAL custom DVE op: (1-pt)^2*bce, accum.
  Morphology: 2-iter erode/dilate == thresholds of W = conv2(m, 13-pt
  diamond). 5 taps computed as 3 fp8 DoubleRow matmuls per slot (two
  vertical-band stationaries per instruction, shifted moving views of the
  padded m). BOUND custom DVE op on PSUM W: (min(W,1)-relu(W-24))*bce, acc.
  Rows 0,1,126,127 at the 3 internal band boundaries of each image get
  wrong W from band truncation; the aux pass (block-diag stationaries
  S5/S3/S1 true + truncated) cancels it exactly via BOUND/NBOUND on 96 rows.
  dice sum(t*pt): GpSimd (otherwise idle) scalar_tensor_tensor
  (t2/32)*pt with accum = Sum((t-0.5)*pt); zh also on GpSimd. sum(t) is
  never needed: Sum(p)+Sum(t) = n - Sum(pt) + 2*Sum(t*pt) (cancellation),
  and Sum(t*pt) = Sum((t-0.5)*pt) + 0.5*Sum(pt).
"""

import numpy as np
import operator

N_CORES = 8
IMG = 8            # images per core
H = 512
W = 512
BANDS = 4          # 128-row bands per image
P = 128
SLOTS = IMG * BANDS   # 32, slot = img*4 + band
GROUPS = 4
GS = SLOTS // GROUPS  # 8 slots per group = 2 images

_CACHE = {}


def _register_dve_op(name, spec):
    from concourse import dve_ops
    from concourse.dve_uop import DveOpSpec
    from concourse.dve_spec import lower
    for op in dve_ops.OPS:
        if op.name == name:
            return op
    opcode = max(dve_ops._SUB_OPCODE_FOR_NAME.values()) + 1
    assert opcode < 0x20
    dve_ops._SUB_OPCODE_FOR_NAME[name] = opcode
    uops = lower(spec, ver="v3")
    sha = DveOpSpec(name=name, opcode=opcode, uops=uops,
                    rd1_en=dve_ops.has_src1(spec)).sha("v3")
    op = dve_ops.DveOp(name, spec, subdim=False, uops_sha={"v3": sha})
    dve_ops.OPS.append(op)
    return op


def _stationaries():
    """Conv stationaries: fp8 DoubleRow pairs + aux block-diag bf16."""
    import ml_dtypes
    bf = ml_dtypes.bfloat16
    f8 = ml_dtypes.float8_e4m3
    kv0 = [1.0, 2.0, 5.0, 2.0, 1.0]   # dc=0 column of the diamond kernel
    kv1 = [2.0, 2.0, 2.0]             # dc=+-1
    B5 = np.zeros((P, P), np.float32)
    B3 = np.zeros((P, P), np.float32)
    for p in range(P):
        for i in range(max(0, p - 2), min(P, p + 3)):
            B5[p, i] = kv0[p - i + 2]
        for i in range(max(0, p - 1), min(P, p + 2)):
            B3[p, i] = kv1[p - i + 1]
    B1 = np.eye(P, dtype=np.float32)
    Z = np.zeros((P, P), np.float32)
    # DoubleRow pairs [K, 2, M]; member i pairs with moving view i.
    # p1: cols (+1, +2) -> (B3*m_-1, B5*m_0); p2: cols (+0, +3) ->
    # (B1*m_-2, B3*m_+1); p3: cols (+0, +4) -> (0, B1*m_+2).
    p1 = np.stack([B3, B5], axis=1)
    p2 = np.stack([B1, B3], axis=1)
    p3 = np.stack([Z, B1], axis=1)
    # aux block-diag: q=(li,b,k ctx row 0..7) -> j=(li,b,w wrong row 0..3)
    # ctx row k = img row 124+128b+k ; wrong row w = img row 126+128b+w
    # vertical delta = k - w - 2
    S5 = np.zeros((96, 48), np.float32)
    S3 = np.zeros((96, 48), np.float32)
    S1 = np.zeros((96, 48), np.float32)
    S5t = np.zeros((96, 48), np.float32)
    S3t = np.zeros((96, 48), np.float32)
    S1t = np.zeros((96, 48), np.float32)
    for li in range(4):
        for b in range(3):
            for k in range(8):
                for w in range(4):
                    d = k - w - 2
                    q = li * 24 + b * 8 + k
                    j = li * 12 + b * 4 + w
                    same = (w < 2 and k < 4) or (w >= 2 and k >= 4)
                    if -2 <= d <= 2:
                        S5[q, j] = kv0[d + 2]
                        if same:
                            S5t[q, j] = kv0[d + 2]
                    if -1 <= d <= 1:
                        S3[q, j] = kv1[d + 1]
                        if same:
                            S3t[q, j] = kv1[d + 1]
                    if d == 0:
                        S1[q, j] = 1.0
                        if same:
                            S1t[q, j] = 1.0
    Z48 = np.zeros((96, 48), np.float32)
    out = {k: v.astype(f8) for k, v in dict(
        p1=p1, p2=p2, p3=p3,
        a1=np.stack([S3, S5], axis=1),
        a2=np.stack([S1, S3], axis=1),
        a3=np.stack([Z48, S1], axis=1),
        a1t=np.stack([S3t, S5t], axis=1),
        a2t=np.stack([S1t, S3t], axis=1),
        a3t=np.stack([Z48, S1t], axis=1)).items()}
    del bf
    return out


def _patch_act_tables():
    """Pin exp/ln/copy activations to the one table set containing all of
    them (natural_log_exp_and_others) so the kernel does a single
    ACT_TABLE_LOAD instead of thrashing between per-function sets."""
    from concourse import bacc as bacc_mod, hw_specs
    orig = hw_specs.get_activation_tables
    keep = "natural_log_exp_and_others"

    def patched(arch):
        t = orig(arch)
        pin = set(t[keep])
        return {k: (v if k == keep else {f for f in v if f not in pin})
                for k, v in t.items()}

    bacc_mod.get_activation_tables = patched
    return lambda: setattr(bacc_mod, "get_activation_tables", orig)


def _build():
    from concourse import bacc, mybir, tile
    from bass_rust import AP
    from concourse.dve_spec import (Spec, Src0, Src1, C0, One, Zero,
                                    minn, maxx, sq)

    f32 = mybir.dt.float32
    bf16 = mybir.dt.bfloat16
    fp8 = mybir.dt.float8e4
    Alu = mybir.AluOpType
    Act = mybir.ActivationFunctionType
    DR = mybir.MatmulPerfMode.DoubleRow

    FOCAL = _register_dve_op("ANT_FOCAL_SSQ", Spec(
        body=sq(One - Src0) * Src1, accum=operator.add))
    BOUND = _register_dve_op("ANT_BOUND_WDF", Spec(
        body=(minn(Src0, One) - maxx(Src0 - C0, Zero)) * Src1,
        accum=operator.add))
    NBOUND = _register_dve_op("ANT_BOUND_NEG", Spec(
        body=(maxx(Src0 - C0, Zero) - minn(Src0, One)) * Src1,
        accum=operator.add))

    unpatch = _patch_act_tables()
    nc = bacc.Bacc("TRN2", target_bir_lowering=False, debug=False,
                   num_devices=N_CORES)

    x_d = nc.dram_tensor("x", [P, SLOTS, W], bf16, kind="ExternalInput").ap()
    t2_d = nc.dram_tensor("t2", [P, SLOTS, W], bf16,
                          kind="ExternalInput").ap()
    m_d = nc.dram_tensor("m", [P, SLOTS, W + 4], fp8,
                         kind="ExternalInput").ap()
    mctx_d = [nc.dram_tensor(f"mctx{h}", [96, W + 4], fp8,
                             kind="ExternalInput").ap() for h in range(2)]
    xwr_d = [nc.dram_tensor(f"xwr{h}", [48, W], bf16,
                            kind="ExternalInput").ap() for h in range(2)]
    twr_d = [nc.dram_tensor(f"twr{h}", [48, W], bf16,
                            kind="ExternalInput").ap() for h in range(2)]
    p1_d = nc.dram_tensor("p1", [P, 2, P], fp8, kind="ExternalInput").ap()
    p2_d = nc.dram_tensor("p2", [P, 2, P], fp8, kind="ExternalInput").ap()
    p3_d = nc.dram_tensor("p3", [P, 2, P], fp8, kind="ExternalInput").ap()
    aux_d = {k: nc.dram_tensor(k, [96, 2, 48], fp8, kind="ExternalInput").ap()
             for k in ("a1", "a2", "a3", "a1t", "a2t", "a3t")}

    # acc cols: 0:4 sum(bce) per group, 4:8 sum(pt), 8:12 focal sum,
    # 12:16 sum((t-0.5)*pt)
    acc_d = nc.dram_tensor("acc", [P, 20], f32, kind="ExternalOutput").ap()
    accb_d = nc.dram_tensor("accb", [P, 8], f32, kind="ExternalOutput").ap()
    acca_d = nc.dram_tensor("acca", [48, 4], f32, kind="ExternalOutput").ap()
    dice_d = nc.dram_tensor("dice", [P, P], f32, kind="ExternalOutput").ap()

    with tile.TileContext(nc) as tc:
        with (
            tc.tile_pool(name="io", bufs=3) as io,
            tc.tile_pool(name="cn", bufs=1) as cn,
            tc.tile_pool(name="ew", bufs=2) as ew,
            tc.tile_pool(name="jk", bufs=1) as jk,
            tc.tile_pool(name="ax", bufs=1) as ax,
            tc.tile_pool(name="psw", bufs=1, space="PSUM") as psw,
            tc.tile_pool(name="psd", bufs=1, space="PSUM") as psd,
            tc.tile_pool(name="psa", bufs=1, space="PSUM") as psa,
        ):
            # chunks: (slot0, nslots); fine-grained at both ends to cut
            # pipeline fill/drain
            CH = [(0, 4), (4, 4), (8, 8), (16, 8), (24, 4), (28, 4)]
            NCH = len(CH)
            tiles = {}

            def conv_rhs(ms, s, c0, step):
                base = ms[:, s, :]
                pd = list(base.ap[0])
                return AP(base.tensor, base.offset + c0,
                          [pd, [step, 2], [1, W]])

            def dma(c):
                s0, ns = CH[c]
                xs = io.tile([P, ns, W], bf16, tag=f"xs{ns}", name=f"xs{c}")
                ts2 = io.tile([P, ns, W], bf16, tag=f"ts{ns}", name=f"ts{c}")
                ms = io.tile([P, ns, W + 4], fp8, tag=f"ms{ns}",
                             name=f"ms{c}")
                h = ns // 2
                nc.sync.dma_start(xs[:, 0:h, :], x_d[:, s0:s0 + h, :])
                nc.sync.dma_start(xs[:, h:ns, :], x_d[:, s0 + h:s0 + ns, :])
                nc.sync.dma_start(ts2[:, 0:h, :], t2_d[:, s0:s0 + h, :])
                nc.sync.dma_start(ts2[:, h:ns, :],
                                  t2_d[:, s0 + h:s0 + ns, :])
                nc.sync.dma_start(ms[:], m_d[:, s0:s0 + ns, :])
                tiles[c] = [xs, ts2, ms]

            def zh_op(c):
                s0, ns = CH[c]
                xs, ts2, ms = tiles[c]
                zh = ew.tile([P, ns, W], bf16, tag=f"zh{ns}", name=f"zh{c}")
                nc.vector.tensor_tensor(out=zh[:], in0=ts2[:], in1=xs[:],
                                        op=Alu.mult)
                tiles[c].append(zh)

            def acts(c):
                s0, ns = CH[c]
                xs, ts2, ms, zh = tiles[c]
                # q overwrites xs (dead after zh) to save SBUF
                q = xs
                nc.scalar.activation(q[:], zh[:], Act.Exp, scale=-0.0625)
                nb = ew.tile([P, ns, W], bf16, tag=f"nb{ns}", name=f"nb{c}")
                nc.scalar.activation(nb[:], q[:], Act.Ln, bias=1.0,
                                     accum_out=acc[:, c:c + 1])
                pt = ew.tile([P, ns, W], bf16, tag=f"pt{ns}", name=f"pt{c}")
                nc.scalar.activation(pt[:], nb[:], Act.Exp, scale=-1.0,
                                     accum_out=acc[:, 6 + c:7 + c])
                tiles[c] += [nb, pt]

            def conv(c, sb_base):
                s0, ns = CH[c]
                ms = tiles[c][2]
                for b in range(ns // 4):
                    wt = psw.tile([P, 4, W], f32, tag="wt",
                                  name=f"W_c{c}_b{b}")
                    for wi, (pw, c0, st) in enumerate(
                            ((p1, 1, 1), (p2, 0, 3), (p3, 0, 4))):
                        for i in range(4):
                            s = b * 4 + i
                            nc.tensor.matmul(wt[:, i, :], pw[:],
                                             conv_rhs(ms, s, c0, st),
                                             start=(wi == 0),
                                             stop=(wi == 2),
                                             perf_mode=DR)
                    tiles[c].append((wt, b, sb_base + b))

            def focal(c):
                s0, ns = CH[c]
                nb, pt = tiles[c][4], tiles[c][5]
                junk2 = jk.tile([P, ns, W], bf16, tag=f"junk2{ns}")
                nc.vector._custom_dve(
                    FOCAL, out=junk2[:], in0=pt[:], in1=nb[:],
                    accum_out=acc[:, 12 + c:13 + c])

            def bound(c):
                s0, ns = CH[c]
                nb = tiles[c][4]
                junk = jk.tile([P, 4, W], bf16, tag="junk")
                for wt, b, col in tiles[c][6:]:
                    nc.vector._custom_dve(
                        BOUND, out=junk[:],
                        in0=wt[:], in1=nb[:, b * 4:(b + 1) * 4, :],
                        s0=24.0, accum_out=accb[:, col:col + 1])

            def diag(c):
                s0, ns = CH[c]
                ts2, pt = tiles[c][1], tiles[c][5]
                for s in range(ns):
                    for k in range(4):
                        nc.tensor.matmul(
                            dice_ps[:],
                            ts2[:, s, k * P:(k + 1) * P],
                            pt[:, s, k * P:(k + 1) * P],
                            start=(c == 0 and s == 0 and k == 0),
                            stop=(c == NCH - 1 and s == ns - 1 and k == 3))

            def aux():
                for h in range(2):
                    mctx, xw, tw = auxt[h]
                    zw = ax.tile([48, W], bf16, tag="zw")
                    nc.vector.scalar_tensor_tensor(
                        out=zw[:], in0=tw[:], scalar=0.5, in1=xw[:],
                        op0=Alu.subtract, op1=Alu.mult)
                    qw = ax.tile([48, W], bf16, tag="qw")
                    bw = ax.tile([48, W], bf16, tag="bw")
                    nc.scalar.activation(qw[:], zw[:], Act.Exp, scale=-2.0)
                    nc.scalar.activation(bw[:], qw[:], Act.Ln, bias=1.0)
                    jw = ax.tile([48, W], bf16, tag="jw")
                    base = mctx[:, :]
                    pd = list(base.ap[0])
                    for op, col, mats in (
                        (BOUND, h, ("a1", "a2", "a3")),
                        (NBOUND, 2 + h, ("a1t", "a2t", "a3t")),
                    ):
                        wt = psa.tile([48, W], f32, tag="wtrue")
                        for wi, (wk, c0, st) in enumerate(
                                ((mats[0], 1, 1), (mats[1], 0, 3),
                                 (mats[2], 0, 4))):
                            rhs = AP(base.tensor, base.offset + c0,
                                     [pd, [st, 2], [1, W]])
                            nc.tensor.matmul(wt[:], auxw[wk][:], rhs,
                                             start=(wi == 0),
                                             stop=(wi == 2),
                                             perf_mode=DR)
                        nc.vector._custom_dve(
                            op, out=jw[:], in0=wt[:], in1=bw[:], s0=24.0,
                            accum_out=acca[:, col:col + 1])

            # ---- emission: chunk-0 data first so its compute can start
            # before the constant/aux/later-chunk DMAs drain ----
            dma(0)

            p1 = cn.tile([P, 2, P], fp8, tag="p1")
            p2 = cn.tile([P, 2, P], fp8, tag="p2")
            p3 = cn.tile([P, 2, P], fp8, tag="p3")
            auxw = {}
            for tl, dd in ((p1, p1_d), (p2, p2_d), (p3, p3_d)):
                nc.sync.dma_start(tl[:], dd[:])
            for k in ("a1", "a2", "a3", "a1t", "a2t", "a3t"):
                auxw[k] = cn.tile([96, 2, 48], fp8, tag=k, name=k)
                nc.sync.dma_start(auxw[k][:], aux_d[k][:])

            # acc cols per chunk c (6 chunks): bce c, pt 6+c, focal 12+c
            acc = cn.tile([P, 20], f32, tag="acc")
            accb = cn.tile([P, 8], f32, tag="accb")
            acca = cn.tile([48, 4], f32, tag="acca")
            nc.vector.memset(acc[:], 0.0)
            nc.vector.memset(accb[:], 0.0)
            nc.vector.memset(acca[:], 0.0)

            dice_ps = psd.tile([P, P], f32, tag="dice")

            dma(1)
            auxt = []
            for h in range(2):
                mctx = ax.tile([96, W + 4], fp8, tag=f"mctx{h}")
                xw = ax.tile([48, W], bf16, tag=f"xw{h}")
                tw = ax.tile([48, W], bf16, tag=f"tw{h}")
                nc.sync.dma_start(mctx[:], mctx_d[h][:])
                nc.sync.dma_start(xw[:], xwr_d[h][:])
                nc.sync.dma_start(tw[:], twr_d[h][:])
                auxt.append((mctx, xw, tw))
            dma(2)
            dma(3)
            dma(4)
            dma(5)

            zh_op(0)
            zh_op(1)
            acts(0)
            conv(0, 0)
            zh_op(2)
            acts(1)
            focal(0)
            bound(0)
            conv(1, 1)
            diag(0)
            acts(2)
            zh_op(3)
            focal(1)
            bound(1)
            conv(2, 2)
            diag(1)
            acts(3)
            zh_op(4)
            focal(2)
            bound(2)
            conv(3, 4)
            diag(2)
            aux()
            acts(4)
            zh_op(5)
            focal(3)
            bound(3)
            conv(4, 6)
            diag(3)
            acts(5)
            focal(4)
            bound(4)
            conv(5, 7)
            diag(4)
            focal(5)
            bound(5)
            diag(5)

            nc.sync.dma_start(acc_d[:], acc[:])
            nc.sync.dma_start(accb_d[:], accb[:])
            nc.sync.dma_start(acca_d[:], acca[:])
            dsb = cn.tile([P, P], f32, tag="dsb")
            nc.scalar.copy(dsb[:], dice_ps[:])
            nc.sync.dma_start(dice_d[:], dsb[:])

    nc.compile()
    unpatch()
    return nc


def _get_nc():
    if "nc" not in _CACHE:
        _CACHE["nc"] = _build()
    return _CACHE["nc"]


def kernel(inputs: np.ndarray, targets: np.ndarray) -> np.ndarray:
    import os
    import ml_dtypes
    from concourse.bass_utils import run_bass_kernel_spmd

    bf = ml_dtypes.bfloat16
    f8 = ml_dtypes.float8_e4m3

    nc = _get_nc()
    st = _stationaries()

    x = np.asarray(inputs, dtype=np.float32).reshape(64, H, W)
    t = np.asarray(targets, dtype=np.float32).reshape(64, H, W)

    in_maps = []
    for c in range(N_CORES):
        xc = x[c * IMG:(c + 1) * IMG]     # [8, 512, 512]
        tc_ = t[c * IMG:(c + 1) * IMG]
        # transposed layout: [128 p=row-in-band, slot=img*4+band, 512]
        xT = np.ascontiguousarray(
            xc.reshape(IMG, BANDS, P, W).transpose(2, 0, 1, 3)
            .reshape(P, SLOTS, W)).astype(bf)
        tT = (tc_.reshape(IMG, BANDS, P, W).transpose(2, 0, 1, 3)
              .reshape(P, SLOTS, W))
        t2 = np.ascontiguousarray(32.0 * (tT - 0.5)).astype(bf)
        mp = np.zeros((P, SLOTS, W + 4), np.float32)
        mp[:, :, 2:2 + W] = tT
        im = {"x": xT, "t2": t2, "m": mp.astype(f8)}
        # aux: ctx rows 124+128b..131+128b, wrong rows 126+128b..129+128b
        for h in range(2):
            imgs = tc_[4 * h:4 * h + 4]
            ximgs = xc[4 * h:4 * h + 4]
            tctx = np.stack([imgs[li, 124 + 128 * b:132 + 128 * b, :]
                             for li in range(4) for b in range(3)])
            mctx = np.zeros((96, W + 4), np.float32)
            mctx[:, 2:2 + W] = tctx.reshape(96, W)
            im[f"mctx{h}"] = mctx.astype(f8)
            twr = np.stack([imgs[li, 126 + 128 * b:130 + 128 * b, :]
                            for li in range(4) for b in range(3)])
            im[f"twr{h}"] = np.ascontiguousarray(
                twr.reshape(48, W)).astype(bf)
            xwr = np.stack([ximgs[li, 126 + 128 * b:130 + 128 * b, :]
                            for li in range(4) for b in range(3)])
            im[f"xwr{h}"] = np.ascontiguousarray(
                xwr.reshape(48, W)).astype(bf)
        im.update(st)
        in_maps.append(im)

    trace = bool(os.environ.get("BASS_TRACE_KERNEL"))
    res = run_bass_kernel_spmd(nc, in_maps, core_ids=list(range(N_CORES)),
                               trace=trace)
    _CACHE["exec_time_ns"] = res.exec_time_ns

    s_bce = s_pt = s_focal = s_bnd = s_diag = 0.0
    for c in range(N_CORES):
        acc = res.results[c]["acc"].astype(np.float64)
        s_bce += acc[:, 0:6].sum()
        s_pt += acc[:, 6:12].sum()
        s_focal += acc[:, 12:18].sum()
        s_bnd += res.results[c]["accb"].astype(np.float64).sum()
        s_bnd += res.results[c]["acca"].astype(np.float64).sum()
        s_diag += np.trace(res.results[c]["dice"].astype(np.float64))

    n = float(64 * H * W)
    s_tpt = (s_diag + 16.0 * s_pt) / 32.0
    focal_loss = 0.25 * s_focal / n
    denom = n - s_pt + 2.0 * s_tpt
    dice = (2.0 * s_tpt + 1e-6) / (denom + 1e-6)
    dice_loss = 1.0 - dice
    boundary_loss = (s_bce + 5.0 * s_bnd) / n
    loss = 0.3 * focal_loss + 0.4 * dice_loss + 0.3 * boundary_loss
    return np.float32(loss)
